# revision 8
# baseline (speedup 1.0000x reference)
"""IrrepsLinear Trainium2 kernel: y = per-irrep-block x @ W / sqrt(mul).

Irreps layout: 256x0e + 128x1o + 64x2e -> blocks of width 256*1, 128*3, 64*5.
Data-parallel over 8 NeuronCores: each core gets 12500 nodes.

Strategy (v5):
  - fp16 DRAM IO halves HBM traffic; fp16 matmuls -> fp32 PSUM -> fp16 evac.
  - Host pre-permutes features so each 128-row K-group is contiguous;
    monolithic per-window load/store DMAs (keeps DMA near peak rate).
  - Window schedule tapered at BOTH ends: small first windows let stores
    enter the DMA stream early (loads+stores run concurrently at ~478 GB/s
    combined), small last windows kill the final-store drain exposure.
  - bufs=3 tile pools so loads are not paced by compute (measured: bufs=2
    throttled mid-run loads to ~230 GB/s).
  - First SP-ring dispatch is x data; weight loads go on the ACT ring.
  - Block2 m-pairs (m0,m1),(m2,m3) via 128x128 block-diag W2 stationary;
    m4's 64-row operand is folded across 128 partitions (two column
    halves), using PE tile_position for the upper-half stationary, so
    every DMA uses all 128 partitions (64-partition DMAs ran at half rate).
  - 8 one-bank PSUM tiles rotate per 512-slice; evac alternates DVE/ACT.
"""

import numpy as np

NCORES = 8
N_TOTAL = 100000
NSH = N_TOTAL // NCORES   # 12500 nodes per core
D = 960
MMW = 512                 # matmul slice width (= one fp32 PSUM bank)

# tapered window schedule (all even; sum = NSH)
WINDOWS = [512, 1024, 2048, 2048, 2048, 2048, 1948, 424, 400]
assert sum(WINDOWS) == NSH and all(w % 2 == 0 for w in WINDOWS)
OFFS = np.concatenate([[0], np.cumsum(WINDOWS)[:-1]]).tolist()

DT_IO = "float16"
_BUILD_CACHE = {}


def _perm():
    p = list(range(256))
    for m in range(3):
        p += [256 + 3 * i + m for i in range(128)]
    for m in range(5):
        p += [640 + 5 * i + m for i in range(64)]
    return np.asarray(p, dtype=np.int64)

_PERM = _perm()


def _build_program():
    import concourse.bass as bass  # noqa: F401
    import concourse.bacc as bacc
    import concourse.mybir as mybir
    import concourse.tile as tile

    key = (DT_IO, MMW, tuple(WINDOWS), "v5")
    if key in _BUILD_CACHE:
        return _BUILD_CACHE[key]

    dt = getattr(mybir.dt, DT_IO)
    f32 = mybir.dt.float32

    nc = bacc.Bacc(
        "TRN2", target_bir_lowering=False, debug=False, enable_asserts=False
    )
    xa = nc.dram_tensor("xa", [128, 7 * NSH], dt, kind="ExternalInput").ap()
    xbd = nc.dram_tensor("xb", [128, NSH // 2], dt, kind="ExternalInput").ap()
    w0 = nc.dram_tensor("w0", [256, 256], dt, kind="ExternalInput").ap()
    w1 = nc.dram_tensor("w1", [128, 128], dt, kind="ExternalInput").ap()
    w2d = nc.dram_tensor("w2d", [128, 128], dt, kind="ExternalInput").ap()
    w2s = nc.dram_tensor("w2s", [128, 64], dt, kind="ExternalInput").ap()
    ya = nc.dram_tensor("ya", [128, 7 * NSH], dt, kind="ExternalOutput").ap()
    ybd = nc.dram_tensor("yb", [128, NSH // 2], dt, kind="ExternalOutput").ap()

    with tile.TileContext(nc) as tc:
        with (
            tc.tile_pool(name="const", bufs=1) as cpool,
            tc.tile_pool(name="xin", bufs=3) as xpool,
            tc.tile_pool(name="xbp", bufs=3) as xbp,
            tc.tile_pool(name="yst", bufs=3) as ypool,
            tc.tile_pool(name="ybp", bufs=3) as ybp,
            tc.tile_pool(name="ps", bufs=8, space="PSUM") as pspool,
        ):
            # issue first x loads before anything else on the SP ring
            first_tiles = {}
            for wi in (0, 1):
                c0, sw = OFFS[wi], WINDOWS[wi]
                xat = xpool.tile([128, 7 * sw], dt, name=f"xa{wi}", tag="xa")
                nc.sync.dma_start(xat[:], xa[:, 7 * c0 : 7 * (c0 + sw)])
                xbt = xbp.tile([128, sw // 2], dt, name=f"xb{wi}", tag="xb")
                nc.sync.dma_start(xbt[:], xbd[:, c0 // 2 : (c0 + sw) // 2])
                first_tiles[wi] = (xat, xbt)

            # weights on the ACT HWDGE ring (idle until first evacs)
            w0t0 = cpool.tile([128, 256], dt, name="w0t0", tag="w0t0")
            nc.scalar.dma_start(w0t0[:], w0[0:128, :])
            w0t1 = cpool.tile([128, 256], dt, name="w0t1", tag="w0t1")
            nc.scalar.dma_start(w0t1[:], w0[128:256, :])
            w1t = cpool.tile([128, 128], dt, name="w1t", tag="w1t")
            nc.scalar.dma_start(w1t[:], w1[:, :])
            w2dt = cpool.tile([128, 128], dt, name="w2dt", tag="w2dt")
            nc.scalar.dma_start(w2dt[:], w2d[:, :])
            # w2s duplicated on partitions 0-63 and 64-127
            w2st = cpool.tile([128, 64], dt, name="w2st", tag="w2st")
            nc.scalar.dma_start(w2st[:], w2s[:, :])

            n_evac = 0

            def evac(dst, src):
                nonlocal n_evac
                n_evac += 1
                if n_evac % 2:
                    nc.vector.tensor_copy(dst, src)
                else:
                    nc.scalar.copy(dst, src)

            for wi, (c0, sw) in enumerate(zip(OFFS, WINDOWS)):
                if wi in first_tiles:
                    xat, xbt = first_tiles[wi]
                else:
                    xat = xpool.tile([128, 7 * sw], dt, name=f"xa{wi}", tag="xa")
                    nc.sync.dma_start(xat[:], xa[:, 7 * c0 : 7 * (c0 + sw)])
                    xbt = xbp.tile([128, sw // 2], dt, name=f"xb{wi}", tag="xb")
                    nc.sync.dma_start(xbt[:], xbd[:, c0 // 2 : (c0 + sw) // 2])
                yat = ypool.tile([128, 7 * sw], dt, name=f"ya{wi}", tag="ya")
                ybt = ybp.tile([128, sw // 2], dt, name=f"yb{wi}", tag="yb")

                half = sw // 2
                slices = [
                    (i * MMW, min((i + 1) * MMW, sw))
                    for i in range((sw + MMW - 1) // MMW)
                ]
                for lo, hi in slices:
                    n = hi - lo

                    def pst(nm):
                        return pspool.tile(
                            [128, MMW], f32, name=f"{nm}_{wi}_{lo}", tag="ps"
                        )

                    # block0: 256x0e (K=256 via 2 accum steps, M=256 via 2 obs)
                    for ob in range(2):
                        ps = pst(f"ps_b0_{ob}")
                        oc = slice(128 * ob, 128 * (ob + 1))
                        nc.tensor.matmul(
                            ps[:, :n], w0t0[:, oc], xat[:, 0 * sw + lo : 0 * sw + hi],
                            start=True, stop=False,
                        )
                        nc.tensor.matmul(
                            ps[:, :n], w0t1[:, oc], xat[:, 1 * sw + lo : 1 * sw + hi],
                            start=False, stop=True,
                        )
                        evac(yat[:, ob * sw + lo : ob * sw + hi], ps[:, :n])

                    # block1: 128x1o, 3 m-components
                    for m in range(3):
                        ps = pst(f"ps_b1_{m}")
                        t = 2 + m
                        nc.tensor.matmul(
                            ps[:, :n], w1t[:], xat[:, t * sw + lo : t * sw + hi],
                            start=True, stop=True,
                        )
                        evac(yat[:, t * sw + lo : t * sw + hi], ps[:, :n])

                    # block2: m-pairs via block-diag W2 (full PE width)
                    for g in range(2):
                        ps = pst(f"ps_b2_{g}")
                        t = 5 + g
                        nc.tensor.matmul(
                            ps[:, :n], w2dt[:], xat[:, t * sw + lo : t * sw + hi],
                            start=True, stop=True,
                        )
                        evac(yat[:, t * sw + lo : t * sw + hi], ps[:, :n])

                    # block2 m=4: folded across partition halves; the piece
                    # in window cols [half, sw) lives on partitions 64-127
                    # of xbt at cols shifted by -half.
                    pieces = []
                    if lo < half:
                        pieces.append((0, lo, min(hi, half)))
                    if hi > half:
                        pieces.append((64, max(lo, half), hi))
                    for pbase, plo, phi in pieces:
                        pn = phi - plo
                        col = slice(plo - (half if pbase else 0),
                                    phi - (half if pbase else 0))
                        ps = pst(f"ps_b2_4_{pbase}")
                        nc.tensor.matmul(
                            ps[pbase : pbase + 64, :pn],
                            w2st[pbase : pbase + 64, :],
                            xbt[pbase : pbase + 64, col],
                            start=True, stop=True,
                        )
                        evac(ybt[pbase : pbase + 64, col],
                             ps[pbase : pbase + 64, :pn])

                # stores on ACT ring; final window stores the wide ya last
                if wi == len(WINDOWS) - 1:
                    nc.scalar.dma_start(ybd[:, c0 // 2 : (c0 + sw) // 2], ybt[:])
                    nc.scalar.dma_start(ya[:, 7 * c0 : 7 * (c0 + sw)], yat[:])
                else:
                    nc.scalar.dma_start(ya[:, 7 * c0 : 7 * (c0 + sw)], yat[:])
                    nc.scalar.dma_start(ybd[:, c0 // 2 : (c0 + sw) // 2], ybt[:])

    nc.compile()
    _BUILD_CACHE[key] = nc
    return nc


TRACE = False
LAST_RESULT = None


def kernel(x, W0, W1, W2):
    from concourse import bass_utils

    nc = _build_program()

    npdt = np.float16 if DT_IO == "float16" else None
    if npdt is None:
        import ml_dtypes
        npdt = ml_dtypes.bfloat16

    w0s = (np.asarray(W0, np.float32) / np.sqrt(256.0)).astype(npdt)
    w1s = (np.asarray(W1, np.float32) / np.sqrt(128.0)).astype(npdt)
    w2 = (np.asarray(W2, np.float32) / np.sqrt(64.0)).astype(npdt)
    w2d = np.zeros((128, 128), dtype=npdt)
    w2d[0:64, 0:64] = w2
    w2d[64:128, 64:128] = w2
    w2dup = np.concatenate([w2, w2], axis=0)  # [128, 64]

    # pack x: feature-permuted; per window: [128, (t, n)] blocks; the m4
    # 64-row block is folded into 128 partitions per window (col halves).
    xh = np.asarray(x)[:, _PERM].astype(npdt)
    A = xh.reshape(NCORES, NSH, D)
    blocks, bblocks = [], []
    for c0, sw in zip(OFFS, WINDOWS):
        blk = A[:, c0 : c0 + sw, :896].reshape(NCORES, sw, 7, 128)
        blocks.append(blk.transpose(0, 3, 2, 1).reshape(NCORES, 128, 7 * sw))
        bb = A[:, c0 : c0 + sw, 896:].transpose(0, 2, 1)  # [NC, 64, sw]
        h = sw // 2
        bblocks.append(np.concatenate([bb[:, :, :h], bb[:, :, h:]], axis=1))
    xa_all = np.ascontiguousarray(np.concatenate(blocks, axis=2))
    xb_all = np.ascontiguousarray(np.concatenate(bblocks, axis=2))

    in_maps = []
    for c in range(NCORES):
        in_maps.append({
            "xa": xa_all[c], "xb": xb_all[c],
            "w0": w0s, "w1": w1s, "w2d": w2d, "w2s": w2dup,
        })

    res = bass_utils.run_bass_kernel_spmd(
        nc, in_maps, core_ids=list(range(NCORES)), trace=TRACE
    )
    global LAST_RESULT
    LAST_RESULT = res

    out = np.empty((N_TOTAL, D), dtype=np.float32)
    Yp = np.empty((NCORES, NSH, D), dtype=np.float32)
    for c in range(NCORES):
        yac = res.results[c]["ya"]    # [128, 7*NSH]
        ybc = res.results[c]["yb"]    # [128, NSH//2]
        for c0, sw in zip(OFFS, WINDOWS):
            blk = yac[:, 7 * c0 : 7 * (c0 + sw)].reshape(128, 7, sw)
            Yp[c, c0 : c0 + sw, :896] = (
                blk.transpose(2, 1, 0).reshape(sw, 896)
            )
            h = sw // 2
            wb = ybc[:, c0 // 2 : (c0 + sw) // 2]       # [128, h]
            Yp[c, c0 : c0 + h, 896:] = wb[:64].T
            Yp[c, c0 + h : c0 + sw, 896:] = wb[64:].T
    out[:, _PERM] = Yp.reshape(N_TOTAL, D)
    return out
